# revision 10
# baseline (speedup 1.0000x reference)
"""Deformable self-attention TRN2 kernel.

Problem (hardcoded shapes): B=2, Lq=4096, S=16384, D=256, H=8, P=4 (32 slots/query).

Sharding: 8 cores; core k handles batch k//4, query rows [(k%4)*1024, (k%4+1)*1024).
Each core holds the full value[b] sequence in its HBM and gathers sampled rows
from it with the SWDGE dma_gather instruction (one 2KB row-pair per slot).

Per-core pipeline (8 tiles of 128 queries):
  A) PE: pos/attn projections (qT chunks as lhsT); DVE/ACT: softmax, floor
     (magic-constant), clip, edge masks -> per-tap combine weights wt0/wt1;
     PE transposes pack weights into block-diagonal lhs matrices and the
     clipped indices into the 16-partition-wrapped int16 stream dma_gather
     expects (replicated to all 8 Q7 core groups via SBUF->SBUF DMA).
  B) dma_gather (4096 idxs x 2KB: value rows [r, r+1] per slot), then the
     combine runs on PE with the gathered tile as the stationary operand:
     attnT[d, l] += G[:, j, tap-half].T @ W{tap}[:, 4j:4j+4]; this directly
     yields the transposed activation needed for the W_out projection
     (fused bias via rank-1 matmul), so no intermediate transposes/copies.
"""

import sys

sys.path.insert(0, "/opt/trn_rl_repo")

import numpy as np

import concourse.bass as bass
import concourse.mybir as mybir
import concourse.tile as tile
from concourse import bacc
from concourse.masks import make_identity

F32 = mybir.dt.float32
I16 = mybir.dt.int16
AX = mybir.AxisListType
OP = mybir.AluOpType
ACTF = mybir.ActivationFunctionType

B, LQ, S, D = 2, 4096, 16384, 256
NHEAD, NPOINT = 8, 4
NSLOT = NHEAD * NPOINT          # 32 sampling slots per query
N_CORES = 8
LQ_SHARD = LQ * B // N_CORES    # 1024 queries per core
QT = 128                        # queries per tile
NT = LQ_SHARD // QT             # 8 tiles
NJ = QT * NSLOT // 128          # 32 gather columns per tile
CPOS = 0.5 * (S - 1)            # grid_sample coord scale
MAGIC = 12582912.0              # 1.5 * 2^23 fp32 round-to-int magic

_CACHE = {}


def build_program(gather_queues=1, g_bufs=2, idx_bufs=2):
    nc = bacc.Bacc("TRN2", target_bir_lowering=False, debug=False)

    def din(name, shape):
        return nc.dram_tensor(name, list(shape), F32, kind="ExternalInput").ap()

    qT = din("qT", [128, 2, LQ_SHARD])
    refy = din("refy", [1, LQ_SHARD])
    value = din("value", [S, D])
    wpos = din("wpos", [128, 2, NSLOT])
    bpos = din("bpos", [1, NSLOT])
    wattn = din("wattn", [128, 2, NSLOT])
    battn = din("battn", [1, NSLOT])
    wout = din("wout", [128, 2, D])
    bout = din("bout", [1, D])
    out_d = nc.dram_tensor("out", [LQ_SHARD, D], F32, kind="ExternalOutput").ap()

    # overlapping row-pair view of value: row r -> value[r:r+2, :] flattened (512 f32)
    val2 = bass.AP(value.tensor, 0, [[D, S - 1], [1, 2 * D]])

    with tile.TileContext(nc) as tc:
        with tc.tile_pool(name="consts", bufs=1) as cpool, \
             tc.tile_pool(name="work", bufs=2) as wpool, \
             tc.tile_pool(name="gbuf", bufs=g_bufs) as gpool, \
             tc.tile_pool(name="ps_proj", bufs=1, space="PSUM") as ps_proj, \
             tc.tile_pool(name="ps_tr", bufs=1, space="PSUM") as ps_tr, \
             tc.tile_pool(name="ps_at", bufs=1, space="PSUM") as ps_at, \
             tc.tile_pool(name="ps_out", bufs=1, space="PSUM") as ps_out:

            # ---- constants / persistent tiles ----
            qT_sb = cpool.tile([128, 2, LQ_SHARD], F32)
            nc.sync.dma_start(out=qT_sb[:], in_=qT[:])
            wpos_sb = cpool.tile([128, 2, NSLOT], F32)
            nc.sync.dma_start(out=wpos_sb[:], in_=wpos[:])
            wattn_sb = cpool.tile([128, 2, NSLOT], F32)
            nc.sync.dma_start(out=wattn_sb[:], in_=wattn[:])
            bpos_sb = cpool.tile([1, NSLOT], F32)
            nc.sync.dma_start(out=bpos_sb[:], in_=bpos[:])
            battn_sb = cpool.tile([1, NSLOT], F32)
            nc.sync.dma_start(out=battn_sb[:], in_=battn[:])
            wout_sb = cpool.tile([128, 2, D], F32)
            nc.sync.dma_start(out=wout_sb[:], in_=wout[:])
            bout_sb = cpool.tile([1, D], F32)
            nc.sync.dma_start(out=bout_sb[:], in_=bout[:])
            refy_sb = cpool.tile([1, LQ_SHARD], F32)
            nc.sync.dma_start(out=refy_sb[:], in_=refy[:])

            refpos = cpool.tile([1, LQ_SHARD], F32)
            # refpos = (refy + 1) * CPOS = refy*CPOS + CPOS
            nc.scalar.activation(refpos[:], refy_sb[:], ACTF.Copy, bias=float(CPOS), scale=float(CPOS))

            ident = cpool.tile([128, 128], F32)
            make_identity(nc, ident[:])
            ones_row = cpool.tile([1, 128], F32)
            nc.vector.memset(ones_row[:], 1.0)
            ones32 = cpool.tile([1, NSLOT], F32)
            nc.vector.memset(ones32[:], 1.0)
            c0 = cpool.tile([128, 1], F32)
            nc.vector.memset(c0[:], 0.0)
            cone = cpool.tile([128, 1], F32)
            nc.vector.memset(cone[:], 1.0)
            cbig = cpool.tile([128, 1], F32)
            nc.vector.memset(cbig[:], MAGIC)
            cm1 = cpool.tile([128, 1], F32)
            nc.vector.memset(cm1[:], -1.0)
            chi = cpool.tile([128, 1], F32)      # S-1: first OOB-high start
            nc.vector.memset(chi[:], float(S - 1))
            cclip = cpool.tile([128, 1], F32)    # S-2: max gather start row
            nc.vector.memset(cclip[:], float(S - 2))

            def bc(t):
                return t[:, :1].to_broadcast([128, NSLOT])

            for t in range(NT):
                lsl = bass.ds(t * QT, QT)

                # ---- phase A: projections (one psum bank: cols 0:32 logits, 32:64 pos) ----
                projps = ps_proj.tile([128, 2 * NSLOT], F32)
                logits_ps = projps[:, 0:NSLOT]
                pos_ps = projps[:, NSLOT:2 * NSLOT]
                nc.tensor.matmul(logits_ps, lhsT=qT_sb[:, 0, lsl], rhs=wattn_sb[:, 0, :], start=True, stop=False)
                nc.tensor.matmul(logits_ps, lhsT=qT_sb[:, 1, lsl], rhs=wattn_sb[:, 1, :], start=False, stop=False)
                nc.tensor.matmul(logits_ps, lhsT=ones_row[:1, :], rhs=battn_sb[:1, :], start=False, stop=True)
                nc.tensor.matmul(pos_ps, lhsT=qT_sb[:, 0, lsl], rhs=wpos_sb[:, 0, :], start=True, stop=False)
                nc.tensor.matmul(pos_ps, lhsT=qT_sb[:, 1, lsl], rhs=wpos_sb[:, 1, :], start=False, stop=False)
                nc.tensor.matmul(pos_ps, lhsT=refpos[:1, lsl], rhs=ones32[:1, :], start=False, stop=False)
                nc.tensor.matmul(pos_ps, lhsT=ones_row[:1, :], rhs=bpos_sb[:1, :], start=False, stop=True)

                # ---- softmax over 32 slots (free dim) ----
                negmax = wpool.tile([128, 1], F32)
                nc.vector.tensor_reduce(negmax[:], logits_ps, AX.X, OP.max, negate=True)
                w = wpool.tile([128, NSLOT], F32)
                sums = wpool.tile([128, 1], F32)
                nc.scalar.activation(w[:], logits_ps, ACTF.Exp, bias=negmax[:, :1], accum_out=sums[:, :1])
                rec = wpool.tile([128, 1], F32)
                nc.vector.reciprocal(rec[:], sums[:])
                nc.vector.tensor_tensor(out=w[:], in0=w[:], in1=rec[:, :1].to_broadcast([128, NSLOT]), op=OP.mult)

                # ---- floor / frac / clip / masks ----
                pos = wpool.tile([128, NSLOT], F32)
                nc.vector.tensor_copy(out=pos[:], in_=pos_ps)
                rnd = wpool.tile([128, NSLOT], F32)
                nc.vector.tensor_tensor(out=rnd[:], in0=pos[:], in1=bc(cbig), op=OP.add)
                nc.vector.tensor_tensor(out=rnd[:], in0=rnd[:], in1=bc(cbig), op=OP.subtract)
                gt = wpool.tile([128, NSLOT], F32)
                nc.vector.tensor_tensor(out=gt[:], in0=rnd[:], in1=pos[:], op=OP.is_gt)
                i0 = wpool.tile([128, NSLOT], F32)
                nc.vector.tensor_tensor(out=i0[:], in0=rnd[:], in1=gt[:], op=OP.subtract)
                w1 = wpool.tile([128, NSLOT], F32)
                nc.vector.tensor_tensor(out=w1[:], in0=pos[:], in1=i0[:], op=OP.subtract)
                r = wpool.tile([128, NSLOT], F32)
                nc.vector.tensor_tensor(out=r[:], in0=i0[:], in1=bc(cclip), op=OP.min)
                nc.vector.tensor_tensor(out=r[:], in0=r[:], in1=bc(c0), op=OP.max)
                mge = wpool.tile([128, NSLOT], F32)
                nc.vector.tensor_tensor(out=mge[:], in0=i0[:], in1=bc(c0), op=OP.is_ge)
                mle = wpool.tile([128, NSLOT], F32)
                nc.vector.tensor_tensor(out=mle[:], in0=i0[:], in1=bc(cclip), op=OP.is_le)
                mmid = wpool.tile([128, NSLOT], F32)
                nc.vector.tensor_tensor(out=mmid[:], in0=mge[:], in1=mle[:], op=OP.mult)
                mlo = wpool.tile([128, NSLOT], F32)
                nc.vector.tensor_tensor(out=mlo[:], in0=i0[:], in1=bc(cm1), op=OP.is_equal)
                mhi = wpool.tile([128, NSLOT], F32)
                nc.vector.tensor_tensor(out=mhi[:], in0=i0[:], in1=bc(chi), op=OP.is_equal)
                onem = wpool.tile([128, NSLOT], F32)
                nc.scalar.activation(onem[:], w1[:], ACTF.Copy, bias=1.0, scale=-1.0)
                # wt0 = w * ((1-w1)*mmid + w1*mlo) ; wt1 = w * (w1*mmid + (1-w1)*mhi)
                ta = wpool.tile([128, NSLOT], F32)
                nc.vector.tensor_tensor(out=ta[:], in0=onem[:], in1=mmid[:], op=OP.mult)
                tb = wpool.tile([128, NSLOT], F32)
                nc.vector.tensor_tensor(out=tb[:], in0=w1[:], in1=mlo[:], op=OP.mult)
                wt0 = wpool.tile([128, NSLOT], F32)
                nc.vector.tensor_tensor(out=wt0[:], in0=ta[:], in1=tb[:], op=OP.add)
                nc.vector.tensor_tensor(out=wt0[:], in0=wt0[:], in1=w[:], op=OP.mult)
                tc_ = wpool.tile([128, NSLOT], F32)
                nc.vector.tensor_tensor(out=tc_[:], in0=w1[:], in1=mmid[:], op=OP.mult)
                td = wpool.tile([128, NSLOT], F32)
                nc.vector.tensor_tensor(out=td[:], in0=onem[:], in1=mhi[:], op=OP.mult)
                wt1 = wpool.tile([128, NSLOT], F32)
                nc.vector.tensor_tensor(out=wt1[:], in0=tc_[:], in1=td[:], op=OP.add)
                nc.vector.tensor_tensor(out=wt1[:], in0=wt1[:], in1=w[:], op=OP.mult)

                # ---- transposes into one psum tile (bases 0/32/64/96) ----
                trw0 = ps_tr.tile([32, 128], F32)
                nc.tensor.transpose(trw0[0:32, :], in_=wt0[:], identity=ident[:])
                trw1 = ps_tr.tile([32, 128], F32)
                nc.tensor.transpose(trw1[0:32, :], in_=wt1[:], identity=ident[:])
                trr0 = ps_tr.tile([16, 128], F32)
                nc.tensor.transpose(trr0[0:16, :], in_=r[:, 0:16], identity=ident[:])
                trr1 = ps_tr.tile([16, 128], F32)
                nc.tensor.transpose(trr1[0:16, :], in_=r[:, 16:32], identity=ident[:])

                # ---- block-diagonal combine-weight matrices ----
                lhs0 = wpool.tile([128, 128], F32)
                nc.vector.memset(lhs0[:], 0.0)
                lhs1 = wpool.tile([128, 128], F32)
                nc.vector.memset(lhs1[:], 0.0)
                for i in range(4):
                    nc.vector.tensor_copy(out=lhs0[32 * i:32 * i + 32, i:128:4], in_=trw0[0:32, i:128:4])
                    nc.vector.tensor_copy(out=lhs1[32 * i:32 * i + 32, i:128:4], in_=trw1[0:32, i:128:4])

                # ---- wrapped int16 index stream + replication to 8 Q7 groups ----
                idxt = gpool.tile([128, 2 * QT], I16, bufs=idx_bufs, name="idxt")
                nc.vector.tensor_copy(out=idxt[0:16, 0:2 * QT:2], in_=trr0[0:16, :])
                nc.vector.tensor_copy(out=idxt[0:16, 1:2 * QT:2], in_=trr1[0:16, :])
                for g in range(1, 8):
                    nc.sync.dma_start(out=idxt[16 * g:16 * g + 16, :], in_=idxt[0:16, :])

                # ---- gather: 4096 slots x (2 rows x 256 f32), in 1024-idx chunks
                # (single_packet caps at 64 descriptors per SDMA engine = 1024 idxs) ----
                G = gpool.tile([128, NJ, 2 * D], F32, name="G")
                for gch in range(4):
                    nc.gpsimd.dma_gather(
                        out_ap=G[:, 8 * gch:8 * gch + 8, :],
                        in_ap=val2,
                        idxs_ap=idxt[:, 64 * gch:64 * gch + 64],
                        num_idxs=1024,
                        num_idxs_reg=1024,
                        elem_size=2 * D,
                        elem_step=D,
                        queue_num=t % gather_queues,
                    )

                # ---- combine on PE: attnT[d, l] accumulation ----
                at0 = ps_at.tile([128, 128], F32)
                at1 = ps_at.tile([128, 128], F32)
                atp = (at0, at1)
                for j in range(NJ):
                    for c in range(2):
                        nc.tensor.matmul(atp[c][:, 4 * j:4 * j + 4],
                                         lhsT=G[:, j, 128 * c:128 * c + 128],
                                         rhs=lhs0[:, 4 * j:4 * j + 4], start=True, stop=False)
                        nc.tensor.matmul(atp[c][:, 4 * j:4 * j + 4],
                                         lhsT=G[:, j, D + 128 * c:D + 128 * c + 128],
                                         rhs=lhs1[:, 4 * j:4 * j + 4], start=False, stop=True)
                attnT = wpool.tile([128, 2, 128], F32)
                nc.vector.tensor_copy(out=attnT[:, 0, :], in_=at0[:])
                nc.vector.tensor_copy(out=attnT[:, 1, :], in_=at1[:])

                # ---- output projection + bias ----
                op_ = ps_out.tile([128, D], F32)
                nc.tensor.matmul(op_[:], lhsT=attnT[:, 0, :], rhs=wout_sb[:, 0, :], start=True, stop=False)
                nc.tensor.matmul(op_[:], lhsT=attnT[:, 1, :], rhs=wout_sb[:, 1, :], start=False, stop=False)
                nc.tensor.matmul(op_[:], lhsT=ones_row[:1, :], rhs=bout_sb[:1, :], start=False, stop=True)
                out_sb = wpool.tile([128, D], F32)
                nc.vector.tensor_copy(out=out_sb[:], in_=op_[:])
                nc.sync.dma_start(out=out_d[bass.ds(t * QT, QT), :], in_=out_sb[:])

    nc.compile()
    return nc


def make_in_maps(query, key, value, reference_points, W_off, b_off, W_attn, b_attn, W_out, b_out):
    query = np.asarray(query, dtype=np.float32)
    value = np.asarray(value, dtype=np.float32)
    reference_points = np.asarray(reference_points, dtype=np.float32)
    W_off = np.asarray(W_off, dtype=np.float32)
    b_off = np.asarray(b_off, dtype=np.float32)
    W_attn = np.asarray(W_attn, dtype=np.float32)
    b_attn = np.asarray(b_attn, dtype=np.float32)
    W_out = np.asarray(W_out, dtype=np.float32)
    b_out = np.asarray(b_out, dtype=np.float32)

    # fold grid_sample coordinate transform into the offset head (y columns only)
    wposf = (W_off[:, 1::2] * CPOS).astype(np.float32)           # [256, 32]
    bposf = (b_off[1::2] * CPOS).astype(np.float32)              # [32]

    def chunked(m, ncols):                                        # [256, n] -> [128, 2, n]
        return np.ascontiguousarray(m.reshape(2, 128, ncols).transpose(1, 0, 2))

    wpos_r = chunked(wposf, NSLOT)
    wattn_r = chunked(W_attn, NSLOT)
    wout_r = chunked(W_out, D)

    in_maps = []
    per_core = LQ // (N_CORES // B)                               # 1024
    for k in range(N_CORES):
        b = k // (N_CORES // B)
        q0 = (k % (N_CORES // B)) * per_core
        qs = query[b, q0:q0 + per_core, :]                        # [1024, 256]
        qT_r = np.ascontiguousarray(qs.T.reshape(2, 128, per_core).transpose(1, 0, 2))
        in_maps.append({
            "qT": qT_r,
            "refy": np.ascontiguousarray(reference_points[b, q0:q0 + per_core, 1][None, :]),
            "value": value[b],
            "wpos": wpos_r,
            "bpos": bposf[None, :],
            "wattn": wattn_r,
            "battn": b_attn[None, :].astype(np.float32),
            "wout": wout_r,
            "bout": b_out[None, :].astype(np.float32),
        })
    return in_maps


def kernel(**inputs) -> np.ndarray:
    from concourse.bass_utils import run_bass_kernel_spmd

    if "nc" not in _CACHE:
        _CACHE["nc"] = build_program()
    nc = _CACHE["nc"]
    in_maps = make_in_maps(**inputs)
    res = run_bass_kernel_spmd(nc, in_maps, list(range(N_CORES)), trace=False)
    shards = [res.results[k]["out"] for k in range(N_CORES)]
    out = np.empty((B, LQ, D), dtype=np.float32)
    per_core = LQ // (N_CORES // B)
    for k in range(N_CORES):
        b = k // (N_CORES // B)
        q0 = (k % (N_CORES // B)) * per_core
        out[b, q0:q0 + per_core, :] = shards[k]
    return out


# revision 24
# speedup vs baseline: 77.6694x; 77.6694x over previous
"""Deformable self-attention TRN2 kernel.

Problem (hardcoded shapes): B=2, Lq=4096, S=16384, D=256, H=8, P=4 (32 slots/query).

Sharding: 8 cores; core k handles batch k//4, query rows [(k%4)*1024, (k%4+1)*1024).
Each core holds the full value[b] sequence in its HBM and gathers sampled rows
from it with the SWDGE dma_gather instruction (one 2KB row-pair per slot).

Per-core pipeline (8 tiles of 128 queries):
  A) PE: pos/attn projections (qT chunks as lhsT); DVE/ACT: softmax, floor
     (magic-constant), clip, edge masks -> per-tap combine weights wt0/wt1;
     PE transposes pack weights into block-diagonal lhs matrices and the
     clipped indices into the 16-partition-wrapped int16 stream dma_gather
     expects (replicated to all 8 Q7 core groups via SBUF->SBUF DMA).
  B) dma_gather (4096 idxs x 2KB: value rows [r, r+1] per slot), then the
     combine runs on PE with the gathered tile as the stationary operand:
     attnT[d, l] += G[:, j, tap-half].T @ W{tap}[:, 4j:4j+4]; this directly
     yields the transposed activation needed for the W_out projection
     (fused bias via rank-1 matmul), so no intermediate transposes/copies.
"""

import sys

sys.path.insert(0, "/opt/trn_rl_repo")

import numpy as np

import concourse.bass as bass
import concourse.mybir as mybir
import concourse.tile as tile
from concourse import bacc
from concourse.masks import make_identity

F32 = mybir.dt.float32
F16 = mybir.dt.float16
I16 = mybir.dt.int16
AX = mybir.AxisListType
OP = mybir.AluOpType
ACTF = mybir.ActivationFunctionType

B, LQ, S, D = 2, 4096, 16384, 256
NHEAD, NPOINT = 8, 4
NSLOT = NHEAD * NPOINT          # 32 sampling slots per query
N_CORES = 8
LQ_SHARD = LQ * B // N_CORES    # 1024 queries per core
QT = 128                        # queries per tile
NT = LQ_SHARD // QT             # 8 tiles
NJ = QT * NSLOT // 128          # 32 gather columns per tile
CPOS = 0.5 * (S - 1)            # grid_sample coord scale
MAGIC = 12582912.0              # 1.5 * 2^23 fp32 round-to-int magic

_CACHE = {}


def build_program(gather_queues=1, g_bufs=3, idx_bufs=2, repeat=1, combine=True):
    nc = bacc.Bacc("TRN2", target_bir_lowering=False, debug=False)

    def din(name, shape):
        return nc.dram_tensor(name, list(shape), F32, kind="ExternalInput").ap()

    qT = din("qT", [128, 2, LQ_SHARD])
    refy = din("refy", [1, LQ_SHARD])
    value = nc.dram_tensor("value", [S, D], F16, kind="ExternalInput").ap()
    wpos = din("wpos", [128, 2, NSLOT])
    bpos = din("bpos", [1, NSLOT])
    wattn = din("wattn", [128, 2, NSLOT])
    battn = din("battn", [1, NSLOT])
    wout = din("wout", [128, 2, D])
    bout = din("bout", [1, D])
    out_d = nc.dram_tensor("out", [LQ_SHARD, D], F32, kind="ExternalOutput").ap()

    # overlapping row-pair view of value: row r -> value[r:r+2, :] flattened (512 f32)
    val2 = bass.AP(value.tensor, 0, [[D, S - 1], [1, 2 * D]])

    with tile.TileContext(nc) as tc:
        with tc.tile_pool(name="consts", bufs=1) as cpool, \
             tc.tile_pool(name="work", bufs=2) as wpool, \
             tc.tile_pool(name="gbuf", bufs=g_bufs) as gpool, \
             tc.tile_pool(name="ps_proj", bufs=1, space="PSUM") as ps_proj, \
             tc.tile_pool(name="ps_tr", bufs=1, space="PSUM") as ps_tr, \
             tc.tile_pool(name="ps_at", bufs=1, space="PSUM") as ps_at, \
             tc.tile_pool(name="ps_out", bufs=1, space="PSUM") as ps_out:

            # ---- constants / persistent tiles ----
            qT_sb = cpool.tile([128, 2, LQ_SHARD], F32)
            nc.sync.dma_start(out=qT_sb[:], in_=qT[:])
            wpos_sb = cpool.tile([128, 2, NSLOT], F32)
            nc.sync.dma_start(out=wpos_sb[:], in_=wpos[:])
            wattn_sb = cpool.tile([128, 2, NSLOT], F32)
            nc.sync.dma_start(out=wattn_sb[:], in_=wattn[:])
            bpos_sb = cpool.tile([1, NSLOT], F32)
            nc.sync.dma_start(out=bpos_sb[:], in_=bpos[:])
            battn_sb = cpool.tile([1, NSLOT], F32)
            nc.sync.dma_start(out=battn_sb[:], in_=battn[:])
            wout_sb = cpool.tile([128, 2, D], F32)
            nc.sync.dma_start(out=wout_sb[:], in_=wout[:])
            bout_sb = cpool.tile([1, D], F32)
            nc.sync.dma_start(out=bout_sb[:], in_=bout[:])
            refy_sb = cpool.tile([1, LQ_SHARD], F32)
            nc.sync.dma_start(out=refy_sb[:], in_=refy[:])

            refpos = cpool.tile([1, LQ_SHARD], F32)
            # refpos = (refy + 1) * CPOS = refy*CPOS + CPOS
            nc.scalar.activation(refpos[:], refy_sb[:], ACTF.Copy, bias=float(CPOS), scale=float(CPOS))

            ident = cpool.tile([128, 128], F32)
            make_identity(nc, ident[:])
            ones_row = cpool.tile([1, 128], F32)
            nc.vector.memset(ones_row[:], 1.0)
            ones32 = cpool.tile([1, NSLOT], F32)
            nc.vector.memset(ones32[:], 1.0)
            c0 = cpool.tile([128, 1], F32)
            nc.vector.memset(c0[:], 0.0)
            cone = cpool.tile([128, 1], F32)
            nc.vector.memset(cone[:], 1.0)
            cbig = cpool.tile([128, 1], F32)
            nc.vector.memset(cbig[:], MAGIC)
            cm1 = cpool.tile([128, 1], F32)
            nc.vector.memset(cm1[:], -1.0)
            chi = cpool.tile([128, 1], F32)      # S-1: first OOB-high start
            nc.vector.memset(chi[:], float(S - 1))
            cclip = cpool.tile([128, 1], F32)    # S-2: max gather start row
            nc.vector.memset(cclip[:], float(S - 2))

            def bc(t):
                return t[:, :1].to_broadcast([128, NSLOT])


            # per-tile persistent phase-A outputs (reused across repeat reps)
            lhs0s = [cpool.tile([128, 128], F16, name=f"lhs0_t{t}") for t in range(NT)]
            lhs1s = [cpool.tile([128, 128], F16, name=f"lhs1_t{t}") for t in range(NT)]
            idxts = [cpool.tile([128, 2 * QT], I16, name=f"idxt_t{t}") for t in range(NT)]
            for t in range(NT):
                nc.vector.memset(lhs0s[t][:], 0.0)
                nc.vector.memset(lhs1s[t][:], 0.0)

            for rep in range(repeat):
                # ---------- loop 1: phase A for all tiles (keeps the PE queue
                # ahead of the gather stream so the SWDGE queue never starves) ----------
                for t in range(NT):
                    lsl = bass.ds(t * QT, QT)
                    projps = ps_proj.tile([128, 2 * NSLOT], F32)
                    logits_ps = projps[:, 0:NSLOT]
                    pos_ps = projps[:, NSLOT:2 * NSLOT]
                    nc.tensor.matmul(logits_ps, lhsT=qT_sb[:, 0, lsl], rhs=wattn_sb[:, 0, :], start=True, stop=False)
                    nc.tensor.matmul(logits_ps, lhsT=qT_sb[:, 1, lsl], rhs=wattn_sb[:, 1, :], start=False, stop=False)
                    nc.tensor.matmul(logits_ps, lhsT=ones_row[:1, :], rhs=battn_sb[:1, :], start=False, stop=True)
                    nc.tensor.matmul(pos_ps, lhsT=qT_sb[:, 0, lsl], rhs=wpos_sb[:, 0, :], start=True, stop=False)
                    nc.tensor.matmul(pos_ps, lhsT=qT_sb[:, 1, lsl], rhs=wpos_sb[:, 1, :], start=False, stop=False)
                    nc.tensor.matmul(pos_ps, lhsT=refpos[:1, lsl], rhs=ones32[:1, :], start=False, stop=False)
                    nc.tensor.matmul(pos_ps, lhsT=ones_row[:1, :], rhs=bpos_sb[:1, :], start=False, stop=True)

                    negmax = wpool.tile([128, 1], F32)
                    nc.vector.tensor_reduce(negmax[:], logits_ps, AX.X, OP.max, negate=True)
                    w = wpool.tile([128, NSLOT], F32)
                    sums = wpool.tile([128, 1], F32)
                    nc.scalar.activation(w[:], logits_ps, ACTF.Exp, bias=negmax[:, :1], accum_out=sums[:, :1])
                    rec = wpool.tile([128, 1], F32)
                    nc.vector.reciprocal(rec[:], sums[:])
                    nc.vector.tensor_tensor(out=w[:], in0=w[:], in1=rec[:, :1].to_broadcast([128, NSLOT]), op=OP.mult)

                    pos = wpool.tile([128, NSLOT], F32)
                    nc.vector.tensor_copy(out=pos[:], in_=pos_ps)
                    rnd = wpool.tile([128, NSLOT], F32)
                    nc.vector.tensor_tensor(out=rnd[:], in0=pos[:], in1=bc(cbig), op=OP.add)
                    nc.vector.tensor_tensor(out=rnd[:], in0=rnd[:], in1=bc(cbig), op=OP.subtract)
                    gt = wpool.tile([128, NSLOT], F32)
                    nc.vector.tensor_tensor(out=gt[:], in0=rnd[:], in1=pos[:], op=OP.is_gt)
                    i0 = wpool.tile([128, NSLOT], F32)
                    nc.vector.tensor_tensor(out=i0[:], in0=rnd[:], in1=gt[:], op=OP.subtract)
                    w1 = wpool.tile([128, NSLOT], F32)
                    nc.vector.tensor_tensor(out=w1[:], in0=pos[:], in1=i0[:], op=OP.subtract)
                    r = wpool.tile([128, NSLOT], F32)
                    nc.vector.tensor_tensor(out=r[:], in0=i0[:], in1=bc(cclip), op=OP.min)
                    nc.vector.tensor_tensor(out=r[:], in0=r[:], in1=bc(c0), op=OP.max)
                    mge = wpool.tile([128, NSLOT], F32)
                    nc.vector.tensor_tensor(out=mge[:], in0=i0[:], in1=bc(c0), op=OP.is_ge)
                    mle = wpool.tile([128, NSLOT], F32)
                    nc.vector.tensor_tensor(out=mle[:], in0=i0[:], in1=bc(cclip), op=OP.is_le)
                    mmid = wpool.tile([128, NSLOT], F32)
                    nc.vector.tensor_tensor(out=mmid[:], in0=mge[:], in1=mle[:], op=OP.mult)
                    mlo = wpool.tile([128, NSLOT], F32)
                    nc.vector.tensor_tensor(out=mlo[:], in0=i0[:], in1=bc(cm1), op=OP.is_equal)
                    mhi = wpool.tile([128, NSLOT], F32)
                    nc.vector.tensor_tensor(out=mhi[:], in0=i0[:], in1=bc(chi), op=OP.is_equal)
                    onem = wpool.tile([128, NSLOT], F32)
                    nc.scalar.activation(onem[:], w1[:], ACTF.Copy, bias=1.0, scale=-1.0)
                    ta = wpool.tile([128, NSLOT], F32)
                    nc.vector.tensor_tensor(out=ta[:], in0=onem[:], in1=mmid[:], op=OP.mult)
                    tb = wpool.tile([128, NSLOT], F32)
                    nc.vector.tensor_tensor(out=tb[:], in0=w1[:], in1=mlo[:], op=OP.mult)
                    wt0 = wpool.tile([128, NSLOT], F32)
                    nc.vector.tensor_tensor(out=wt0[:], in0=ta[:], in1=tb[:], op=OP.add)
                    nc.vector.tensor_tensor(out=wt0[:], in0=wt0[:], in1=w[:], op=OP.mult)
                    tc_ = wpool.tile([128, NSLOT], F32)
                    nc.vector.tensor_tensor(out=tc_[:], in0=w1[:], in1=mmid[:], op=OP.mult)
                    td = wpool.tile([128, NSLOT], F32)
                    nc.vector.tensor_tensor(out=td[:], in0=onem[:], in1=mhi[:], op=OP.mult)
                    wt1 = wpool.tile([128, NSLOT], F32)
                    nc.vector.tensor_tensor(out=wt1[:], in0=tc_[:], in1=td[:], op=OP.add)
                    nc.vector.tensor_tensor(out=wt1[:], in0=wt1[:], in1=w[:], op=OP.mult)

                    trw0 = ps_tr.tile([32, 128], F32)
                    nc.tensor.transpose(trw0[0:32, :], in_=wt0[:], identity=ident[:])
                    trw1 = ps_tr.tile([32, 128], F32)
                    nc.tensor.transpose(trw1[0:32, :], in_=wt1[:], identity=ident[:])
                    trr0 = ps_tr.tile([16, 128], F32)
                    nc.tensor.transpose(trr0[0:16, :], in_=r[:, 0:16], identity=ident[:])
                    trr1 = ps_tr.tile([16, 128], F32)
                    nc.tensor.transpose(trr1[0:16, :], in_=r[:, 16:32], identity=ident[:])

                    lhs0, lhs1, idxt = lhs0s[t], lhs1s[t], idxts[t]
                    for i in range(4):
                        nc.vector.tensor_copy(out=lhs0[32 * i:32 * i + 32, i:128:4], in_=trw0[0:32, i:128:4])
                        nc.vector.tensor_copy(out=lhs1[32 * i:32 * i + 32, i:128:4], in_=trw1[0:32, i:128:4])
                    nc.vector.tensor_copy(out=idxt[0:16, 0:2 * QT:2], in_=trr0[0:16, :])
                    nc.vector.tensor_copy(out=idxt[0:16, 1:2 * QT:2], in_=trr1[0:16, :])
                    for g in range(1, 8):
                        nc.sync.dma_start(out=idxt[16 * g:16 * g + 16, :], in_=idxt[0:16, :])

                # ---------- loop 2: gather + combine + output ----------
                for t in range(NT):
                    lhs0, lhs1, idxt = lhs0s[t], lhs1s[t], idxts[t]
                    G = gpool.tile([128, NJ, 2 * D], F16, name="G")
                    for gch in range(4):
                        nc.gpsimd.dma_gather(
                            out_ap=G[:, 8 * gch:8 * gch + 8, :],
                            in_ap=val2,
                            idxs_ap=idxt[:, 64 * gch:64 * gch + 64],
                            num_idxs=1024,
                            num_idxs_reg=1024,
                            elem_size=2 * D,
                            elem_step=D,
                            queue_num=t % gather_queues,
                        )

                    at0 = ps_at.tile([128, 128], F32)
                    at1 = ps_at.tile([128, 128], F32)
                    atp = (at0, at1)
                    njs = NJ if combine else 1
                    for j in range(njs):
                        for c in range(2):
                            nc.tensor.matmul(atp[c][:, 4 * j:4 * j + 4],
                                             lhsT=G[:, j, 128 * c:128 * c + 128],
                                             rhs=lhs0[:, 4 * j:4 * j + 4], start=True, stop=False)
                            nc.tensor.matmul(atp[c][:, 4 * j:4 * j + 4],
                                             lhsT=G[:, j, D + 128 * c:D + 128 * c + 128],
                                             rhs=lhs1[:, 4 * j:4 * j + 4], start=False, stop=True)
                    attnT = wpool.tile([128, 2, 128], F32)
                    nc.vector.tensor_copy(out=attnT[:, 0, :], in_=at0[:])
                    nc.vector.tensor_copy(out=attnT[:, 1, :], in_=at1[:])

                    op_ = ps_out.tile([128, D], F32)
                    nc.tensor.matmul(op_[:], lhsT=attnT[:, 0, :], rhs=wout_sb[:, 0, :], start=True, stop=False)
                    nc.tensor.matmul(op_[:], lhsT=attnT[:, 1, :], rhs=wout_sb[:, 1, :], start=False, stop=False)
                    nc.tensor.matmul(op_[:], lhsT=ones_row[:1, :], rhs=bout_sb[:1, :], start=False, stop=True)
                    out_sb = wpool.tile([128, D], F32)
                    nc.vector.tensor_copy(out=out_sb[:], in_=op_[:])
                    nc.sync.dma_start(out=out_d[bass.ds(t * QT, QT), :], in_=out_sb[:])

    nc.compile()
    return nc


def make_in_maps(query, key, value, reference_points, W_off, b_off, W_attn, b_attn, W_out, b_out):
    query = np.asarray(query, dtype=np.float32)
    value = np.asarray(value, dtype=np.float32)
    reference_points = np.asarray(reference_points, dtype=np.float32)
    W_off = np.asarray(W_off, dtype=np.float32)
    b_off = np.asarray(b_off, dtype=np.float32)
    W_attn = np.asarray(W_attn, dtype=np.float32)
    b_attn = np.asarray(b_attn, dtype=np.float32)
    W_out = np.asarray(W_out, dtype=np.float32)
    b_out = np.asarray(b_out, dtype=np.float32)

    # fold grid_sample coordinate transform into the offset head (y columns only)
    wposf = (W_off[:, 1::2] * CPOS).astype(np.float32)           # [256, 32]
    bposf = (b_off[1::2] * CPOS).astype(np.float32)              # [32]

    def chunked(m, ncols):                                        # [256, n] -> [128, 2, n]
        return np.ascontiguousarray(m.reshape(2, 128, ncols).transpose(1, 0, 2))

    wpos_r = chunked(wposf, NSLOT)
    wattn_r = chunked(W_attn, NSLOT)
    wout_r = chunked(W_out, D)

    in_maps = []
    per_core = LQ // (N_CORES // B)                               # 1024
    for k in range(N_CORES):
        b = k // (N_CORES // B)
        q0 = (k % (N_CORES // B)) * per_core
        qs = query[b, q0:q0 + per_core, :]                        # [1024, 256]
        qT_r = np.ascontiguousarray(qs.T.reshape(2, 128, per_core).transpose(1, 0, 2))
        in_maps.append({
            "qT": qT_r,
            "refy": np.ascontiguousarray(reference_points[b, q0:q0 + per_core, 1][None, :]),
            "value": value[b].astype(np.float16),
            "wpos": wpos_r,
            "bpos": bposf[None, :],
            "wattn": wattn_r,
            "battn": b_attn[None, :].astype(np.float32),
            "wout": wout_r,
            "bout": b_out[None, :].astype(np.float32),
        })
    return in_maps


def kernel(**inputs) -> np.ndarray:
    from concourse.bass_utils import run_bass_kernel_spmd

    if "nc" not in _CACHE:
        _CACHE["nc"] = build_program()
    nc = _CACHE["nc"]
    in_maps = make_in_maps(**inputs)
    res = run_bass_kernel_spmd(nc, in_maps, list(range(N_CORES)), trace=False)
    shards = [res.results[k]["out"] for k in range(N_CORES)]
    out = np.empty((B, LQ, D), dtype=np.float32)
    per_core = LQ // (N_CORES // B)
    for k in range(N_CORES):
        b = k // (N_CORES // B)
        q0 = (k % (N_CORES // B)) * per_core
        out[b, q0:q0 + per_core, :] = shards[k]
    return out
